# revision 1
# baseline (speedup 1.0000x reference)
"""Trainium2 Bass kernel for nn_CLUBCategorical (CLUB categorical loss).

Reference computation:
    h      = relu(x @ W1 + b1)              [N, H]
    logits = h @ W2 + b2                    [N, Y]
    logp   = log_softmax(logits, -1)        [N, Y]
    out[i] = logp[i, y_i] - mean_j logp[i, y_j]

Algebraic simplification: with c[y] = histogram(y_idx), the log-softmax
normalizer cancels between the positive and negative terms:

    out[i] = L[i, y_i] - (1/N) * (L[i, :] @ c) + (b2[y_i] - (b2 @ c)/N)

where L = relu(x @ W1 + b1) @ W2 (no bias, no softmax). On device this is
two dense matmuls plus a masked column reduction:

    out[i] = sum_y L[i, y] * (onehot(y_i)[y] - c[y]/N) + g[i]

Sharding: data-parallel over N. Each of the 8 cores handles 1024 rows and
gets the full W1/W2 plus the global label histogram (the "all-gather of
column labels" is folded into c on the host). No collectives needed.

Device layout (per core; contraction dim always on SBUF partitions, all
operand layouts pre-arranged on host so every DMA is one contiguous
descriptor):
    phase 1: hT[m]  [128h, 1024r] = W1[:,mslice].T @ xT[:, rows] (+b1, relu)
    phase 2: psum_l [128y,  512r] = W2[:,qslice].T @ hT[:, rows]
             eqc    [128y,  512r] = (ybc == iota_q) - cN_q       (DVE)
             prod   = psum_l * eqc                               (DVE)
             out    += ones.T @ prod  (M=1 matmul reduces over y) (PE)
ybc is broadcast on device from a [1, rows] vector via a K=1 matmul.
Matmuls run in float32r (~2^-13 relative precision, 2x fp32 throughput).
DMA descriptors are interleaved across the two HWDGE queues (sync,
scalar) in phase-1 consumption order; constants ride the gpsimd SWDGE.
"""

import numpy as np

N, X_DIM, Y_DIM, HIDDEN = 8192, 512, 512, 1024
N_CORES = 8
N_LOC = N // N_CORES          # 1024 rows per core
KX = X_DIM // 128             # 4  k-chunks, phase 1
KH = HIDDEN // 128            # 8  k-chunks, phase 2 / m-chunks, phase 1
QY = Y_DIM // 128             # 4  y-chunks, phase 2
RG = N_LOC // 512             # 2  row groups of 512

_NC_CACHE = {}


def _build(nc_cls, mybir, tile):
    mdt = mybir.dt
    f32 = mdt.float32
    F32R = mdt.float32r
    AF = mybir.ActivationFunctionType
    OP = mybir.AluOpType

    nc = nc_cls("TRN2", target_bir_lowering=False, debug=False,
                num_devices=N_CORES)

    # xt{n}{a,b}: x rows for row-group n, partition-major, k-halves
    xtD = [[nc.dram_tensor(f"xt{n}{h}", [128, 2 * 512], f32,
                           kind="ExternalInput") for h in "ab"]
           for n in range(RG)]
    # w1p{mp}: W1 columns for hidden-pair mp, all K
    w1D = [nc.dram_tensor(f"w1p{mp}", [128, KX * 256], f32,
                          kind="ExternalInput") for mp in range(KH // 2)]
    # w2p{h}: W2 rows half h, partition-major
    w2D = [nc.dram_tensor(f"w2p{h}", [128, 4 * Y_DIM], f32,
                          kind="ExternalInput") for h in range(2)]
    # packed constants: [b1c(8) | iot(4) | cNc(4) | ones(1)] = [128, 17]
    cst = nc.dram_tensor("cst", [128, KH + 2 * QY + 1], f32,
                         kind="ExternalInput")
    o128 = nc.dram_tensor("o128", [1, 128], f32, kind="ExternalInput")
    yrow = nc.dram_tensor("yrow", [1, N_LOC], f32, kind="ExternalInput")
    gv = nc.dram_tensor("gv", [1, N_LOC], f32, kind="ExternalInput")
    out = nc.dram_tensor("out", [1, N_LOC], f32, kind="ExternalOutput")

    with tile.TileContext(nc) as tc:
        with (
            tc.tile_pool(name="wgt", bufs=1) as wgt,
            tc.tile_pool(name="hp", bufs=1) as hp,
            tc.tile_pool(name="eqp", bufs=1) as eqp,
            tc.tile_pool(name="prp", bufs=4) as prp,
            tc.tile_pool(name="osb", bufs=1) as osb,
            tc.tile_pool(name="ps", bufs=1, space="PSUM") as ps,
        ):
            cst_sb = wgt.tile([128, KH + 2 * QY + 1], F32R, tag="cst")
            b1_sb = cst_sb[:, 0:KH].bitcast(f32)
            iot_sb = cst_sb[:, KH:KH + QY].bitcast(f32)
            cnc_sb = cst_sb[:, KH + QY:KH + 2 * QY].bitcast(f32)
            ones_sb = cst_sb[:, KH + 2 * QY:KH + 2 * QY + 1]
            yrow_sb = wgt.tile([1, N_LOC], F32R, tag="yrow")
            o128_sb = wgt.tile([1, 128], F32R, tag="o128")
            g_sb = wgt.tile([1, N_LOC], f32, tag="g")
            nc.gpsimd.dma_start(g_sb[:], gv.ap())

            # --- big loads, interleaved across both HWDGE queues in
            # phase-1 consumption order ---
            xt_sb = [wgt.tile([128, KX * 512], F32R, tag=f"xt_{n}",
                              name=f"xt_{n}") for n in range(RG)]
            w1p_sb = [wgt.tile([128, KX * 256], F32R, tag=f"w1_{mp}",
                               name=f"w1_{mp}") for mp in range(KH // 2)]
            w2p_sb = [wgt.tile([128, 4 * Y_DIM], F32R, tag=f"w2p_{h}",
                               name=f"w2p_{h}") for h in range(2)]
            # Arrival schedule (each queue ~166GB/s, ~3us per 512KB):
            # sync:   xt0a@12 w1p1@15 w1p3@18 w2p0a@21 xt1a@24 w2p1a@27
            # scalar: xt0b@12 w1p0@12+ w1p2@18 w2p0b@21 xt1b@24 w2p1b@27
            # matching PE consumption: p1(n0) -> p2(n0) j0..3 -> p1(n1)
            # -> p2(n0) j4..7 -> p2(n1); only 16 MMs depend on the last MB.
            nc.sync.dma_start(xt_sb[0][:, 0:1024],
                              xtD[0][0].ap().bitcast(F32R))
            nc.scalar.dma_start(w1p_sb[0][:], w1D[0].ap().bitcast(F32R))
            nc.scalar.dma_start(xt_sb[0][:, 1024:2048],
                                xtD[0][1].ap().bitcast(F32R))
            nc.sync.dma_start(yrow_sb[:], yrow.ap().bitcast(F32R))
            nc.sync.dma_start(o128_sb[:], o128.ap().bitcast(F32R))
            nc.scalar.dma_start(cst_sb[:], cst.ap().bitcast(F32R))
            nc.sync.dma_start(w1p_sb[1][:], w1D[1].ap().bitcast(F32R))
            nc.scalar.dma_start(w1p_sb[2][:], w1D[2].ap().bitcast(F32R))
            nc.sync.dma_start(w1p_sb[3][:], w1D[3].ap().bitcast(F32R))
            nc.sync.dma_start(w2p_sb[0][:, 0:1024],
                              w2D[0].ap()[:, 0:1024].bitcast(F32R))
            nc.scalar.dma_start(w2p_sb[0][:, 1024:2048],
                                w2D[0].ap()[:, 1024:2048].bitcast(F32R))
            nc.sync.dma_start(xt_sb[1][:, 0:1024],
                              xtD[1][0].ap().bitcast(F32R))
            nc.scalar.dma_start(xt_sb[1][:, 1024:2048],
                                xtD[1][1].ap().bitcast(F32R))
            nc.sync.dma_start(w2p_sb[1][:, 0:1024],
                              w2D[1].ap()[:, 0:1024].bitcast(F32R))
            nc.scalar.dma_start(w2p_sb[1][:, 1024:2048],
                                w2D[1].ap()[:, 1024:2048].bitcast(F32R))
            w2_sb = [w2p_sb[j // 4][:, (j % 4) * Y_DIM:(j % 4 + 1) * Y_DIM]
                     for j in range(KH)]

            def w1_slice(k, m):
                mp, mo = m // 2, m % 2
                return w1p_sb[mp][:, k * 256 + mo * 128:
                                  k * 256 + (mo + 1) * 128]

            def xt_slice(k, n):
                return xt_sb[n][:, k * 512:(k + 1) * 512]

            # pout accumulators reuse the yb slots (free after eqc)
            pout = {n: ps.tile([1, 512], f32, tag="yb", bufs=RG,
                               name=f"po_{n}") for n in range(RG)}

            hT = [hp.tile([128, N_LOC], F32R, tag=f"h_{j}", name=f"h_{j}")
                  for j in range(KH)]

            def phase1(n):
                for m in range(KH):
                    psum = ps.tile([128, 512], f32, tag="psum", bufs=6,
                                   name=f"p1_{n}_{m}")
                    for k in range(KX):
                        nc.tensor.matmul(
                            psum[:], w1_slice(k, m), xt_slice(k, n),
                            start=(k == 0), stop=(k == KX - 1))
                    nc.scalar.activation(
                        hT[m][:, n * 512:(n + 1) * 512], psum[:],
                        AF.Relu, bias=b1_sb[:, m:m + 1])

            def p2_mms(psum_l, n, q, j0, j1):
                for j in range(j0, j1):
                    nc.tensor.matmul(
                        psum_l[:],
                        w2_sb[j][:, q * 128:(q + 1) * 128],
                        hT[j][:, n * 512:(n + 1) * 512],
                        start=(j == 0), stop=(j == KH - 1))

            pending = []  # delay ones-MMs so PE never waits on DVE prod

            def finish_group(psum_l, n, q):
                prod = prp.tile([128, 512], F32R, name=f"prod_{n}_{q}")
                nc.vector.tensor_tensor(
                    prod[:], psum_l[:], eqc_sb[(n, q)][:], OP.mult)
                pending.append((n, q, prod))

            def flush_one():
                n, q, prod = pending.pop(0)
                nc.tensor.matmul(
                    pout[n][:], ones_sb, prod[:],
                    start=(q == 0), stop=(q == QY - 1))

            # pipelined schedule matched to DMA arrivals.
            # First: 8 K=128 warmup matmuls on a self-produced tile with
            # ZERO DMA dependencies (memset -> DVE round to f32r), so they
            # run during the otherwise-dead preamble window and the PE is
            # already at the warm 2.4GHz clock when the first real
            # operands land.
            # warmup source produced on-chip (memset -> DVE round to
            # f32r): no DMA dependency, so the warmup matmuls run during
            # the otherwise-dead preamble/DMA-lead-in window
            wu_f32 = wgt.tile([128, 512], f32, tag="wuf")
            nc.gpsimd.memset(wu_f32[:], 1.0)
            wu_src = wgt.tile([128, 512], F32R, tag="wur")
            nc.vector.tensor_copy(wu_src[:], wu_f32[:])
            wu = ps.tile([128, 512], f32, tag="psum", bufs=6, name="wu")
            for _ in range(8):
                nc.tensor.matmul(wu[:], wu_src[:, 0:128],
                                 wu_src[:], start=True, stop=True)
            phase1(0)
            # ybc broadcast: K=1 matmul replicates yrow across partitions;
            # eqc masks read it straight from PSUM (needed at finish_group)
            eqc_sb = {}
            for n in range(RG):
                yb = ps.tile([128, 512], f32, tag="yb", bufs=RG,
                             name=f"yb{n}")
                nc.tensor.matmul(
                    yb[:], o128_sb[:],
                    yrow_sb[:, n * 512:(n + 1) * 512],
                    start=True, stop=True)
                for q in range(QY):
                    e = eqp.tile([128, 512], f32, tag=f"eqc_{n}_{q}",
                                 name=f"eqc_{n}_{q}")
                    nc.vector.tensor_scalar(
                        e[:], yb[:], iot_sb[:, q:q + 1], cnc_sb[:, q:q + 1],
                        OP.is_equal, OP.subtract)
                    eqc_sb[(n, q)] = e
            pl_n0 = [ps.tile([128, 512], f32, tag="psum", bufs=6,
                             name=f"pl_0_{q}") for q in range(QY)]
            for q in range(QY):           # needs only w2p0 (j0..3)
                p2_mms(pl_n0[q], 0, q, 0, KH // 2)
            phase1(1)                     # needs xt1; w2p1 streams behind
            for q in range(QY):           # finish n0 with w2p1 (j4..7)
                p2_mms(pl_n0[q], 0, q, KH // 2, KH)
                finish_group(pl_n0[q], 0, q)
            for q in range(QY):
                psum_l = ps.tile([128, 512], f32, tag="psum", bufs=6,
                                 name=f"pl_1_{q}")
                p2_mms(psum_l, 1, q, 0, KH)
                finish_group(psum_l, 1, q)
                flush_one()
            while pending:
                flush_one()

            # --- epilogue: add g, store (single output DMA) ---
            o = osb.tile([1, N_LOC], f32, tag="o")
            for n in range(RG):
                nc.vector.tensor_tensor(
                    o[:, n * 512:(n + 1) * 512], pout[n][:],
                    g_sb[:, n * 512:(n + 1) * 512], OP.add)
            nc.sync.dma_start(out.ap(), o[:])

    nc.compile()
    return nc


def _get_nc():
    if "nc" not in _NC_CACHE:
        import concourse.bacc as bacc
        import concourse.mybir as mybir
        from concourse import tile
        _NC_CACHE["nc"] = _build(bacc.Bacc, mybir, tile)
    return _NC_CACHE["nc"]


def kernel(x_samples, y_idx, W1, b1, W2, b2):
    from concourse.bass_utils import run_bass_kernel_spmd

    x = np.ascontiguousarray(np.asarray(x_samples, dtype=np.float32))
    y = np.asarray(y_idx).astype(np.int64).reshape(-1)
    W1 = np.ascontiguousarray(np.asarray(W1, dtype=np.float32))
    b1 = np.asarray(b1, dtype=np.float32).reshape(-1)
    W2 = np.ascontiguousarray(np.asarray(W2, dtype=np.float32))
    b2 = np.asarray(b2, dtype=np.float32).reshape(-1)

    # global label histogram + fully-folded bias term
    c = np.bincount(y, minlength=Y_DIM).astype(np.float32)
    cN = c / np.float32(N)
    beta = np.float32(b2 @ c) / np.float32(N)
    g_full = (b2[y] - beta).astype(np.float32)

    # device layouts: every DMA is one contiguous descriptor
    # w1_dev[mp][p, k*256+c] = W1[k*128+p, mp*256+c]
    w1_dev = np.ascontiguousarray(
        W1.reshape(KX, 128, KH // 2, 256).transpose(2, 1, 0, 3)
        .reshape(KH // 2, 128, KX * 256))
    # w2_dev[h][p, a*512+y] = W2[(h*4+a)*128+p, y]
    w2_dev = np.ascontiguousarray(
        W2.reshape(2, 4, 128, Y_DIM).transpose(0, 2, 1, 3)
        .reshape(2, 128, 4 * Y_DIM))
    b1c = b1.reshape(KH, 128).T                                   # [128, 8]
    iot = np.arange(Y_DIM, dtype=np.float32).reshape(QY, 128).T   # [128, 4]
    cNc = cN.reshape(QY, 128).T                                   # [128, 4]
    onesv = np.ones((128, 1), dtype=np.float32)
    cst = np.ascontiguousarray(
        np.concatenate([b1c, iot, cNc, onesv], axis=1))           # [128, 17]
    o128 = np.ones((1, 128), dtype=np.float32)

    in_maps = []
    for m in range(N_CORES):
        sl = slice(m * N_LOC, (m + 1) * N_LOC)
        # xt_dev[n][p, k*512+r] = x[m*N_LOC + n*512+r, k*128+p]
        xt_dev = np.ascontiguousarray(
            x[sl].reshape(RG, 512, KX, 128).transpose(0, 3, 2, 1)
            .reshape(RG, 128, KX * 512))
        im = {
            **{f"w1p{mp}": w1_dev[mp] for mp in range(KH // 2)},
            **{f"w2p{h}": w2_dev[h] for h in range(2)},
            "cst": cst,
            "o128": o128,
            "yrow": np.ascontiguousarray(
                y[sl].astype(np.float32)).reshape(1, N_LOC),
            "gv": np.ascontiguousarray(g_full[sl]).reshape(1, N_LOC),
        }
        for n in range(RG):
            im[f"xt{n}a"] = np.ascontiguousarray(xt_dev[n][:, 0:1024])
            im[f"xt{n}b"] = np.ascontiguousarray(xt_dev[n][:, 1024:2048])
        in_maps.append(im)

    nc = _get_nc()
    res = run_bass_kernel_spmd(nc, in_maps, core_ids=list(range(N_CORES)))
    return np.concatenate(
        [res.results[m]["out"].reshape(-1) for m in range(N_CORES)]
    ).astype(np.float32)



# revision 8
# speedup vs baseline: 1.1887x; 1.1887x over previous
"""Trainium2 Bass kernel for nn_CLUBCategorical (CLUB categorical loss).

Reference computation:
    h      = relu(x @ W1 + b1)              [N, H]
    logits = h @ W2 + b2                    [N, Y]
    logp   = log_softmax(logits, -1)        [N, Y]
    out[i] = logp[i, y_i] - mean_j logp[i, y_j]

Algebra: the log-softmax normalizer cancels between the positive and
negative terms, and with c[y] = histogram(y_idx), v = W2 @ c / N:

    out[i] = h_i . (W2[:, y_i] - v) + (b2[y_i] - (b2 . c)/N)
           = h_i . U[:, i] + g[i]

so the entire [N, H] x [H, Y] second GEMM collapses to an elementwise
multiply with the host-gathered U plus a free-dim reduction. Per core
(1024 rows) the device work is just the phase-1 GEMM:

    psum[128 rows, 512 h] = b1 (K=1 ones-matmul) + sum_k xT_blk @ W1_k
    hr = relu(psum)                     (scalar engine, bf16 out)
    delta[128,1] = reduce_h(hr * U_b) + g_b   (one fused DVE
                   tensor_tensor_reduce per 128-row block)

All matmul/elementwise operands are bf16 (PE runs 1 col/cycle at fp32r
and bf16 alike, but bf16 halves DMA to 4MB/core; tolerance is 2e-2 and
bf16 end-to-end lands ~5e-3). Rows are data-parallel across 8 cores; the
"all-gather of column labels" is folded into c/U/g on the host. No
collectives.

Schedule: the 8 sweep-1 bias matmuls are emitted before any data matmul
so the PE ramps its clock and does useful work during the DMA lead-in.
DMA rides two HWDGE queues (sync: cst/W1/U0-3/g, vector: x-blocks/U4-7)
ordered in consumption order.
"""

import numpy as np

N, X_DIM, Y_DIM, HIDDEN = 8192, 512, 512, 1024
N_CORES = 8
N_LOC = N // N_CORES          # 1024 rows per core
NB = N_LOC // 128             # 8 row blocks of 128
KX = X_DIM // 128             # 4 k-chunks
HH = 2                        # two 512-wide hidden halves

_NC_CACHE = {}


def _build(nc_cls, mybir, tile):
    mdt = mybir.dt
    f32 = mdt.float32
    bf16 = mdt.bfloat16
    AF = mybir.ActivationFunctionType
    OP = mybir.AluOpType

    nc = nc_cls("TRN2", target_bir_lowering=False, debug=False,
                num_devices=N_CORES)

    # DRAM tensors (all contiguous, one DMA descriptor each)
    # x blocks fused into 3 descriptors (b0-2, b3-5, b6-7)
    xqD = [nc.dram_tensor(f"xq{i}", [128, w], bf16, kind="ExternalInput")
           for i, w in enumerate((1536, 1536, 1024))]
    # w1a: h-half 0, k 0-1; w1b: h-half 0, k 2-3; w1c: h-half 1, all k
    w1aD = nc.dram_tensor("w1a", [128, 1024], bf16, kind="ExternalInput")
    w1bD = nc.dram_tensor("w1b", [128, 1024], bf16, kind="ExternalInput")
    w1cD = nc.dram_tensor("w1c", [128, 2048], bf16, kind="ExternalInput")
    # U in 4 descriptors of 2 row-blocks each
    uqD = [nc.dram_tensor(f"uq{i}", [128, 2048], bf16, kind="ExternalInput")
           for i in range(4)]
    cstD = nc.dram_tensor("cst", [1, HIDDEN + 128], bf16,
                          kind="ExternalInput")   # [b1 | ones128]
    gtD = nc.dram_tensor("gt", [128, NB], f32, kind="ExternalInput")
    outD = nc.dram_tensor("out", [128, NB], f32, kind="ExternalOutput")

    with tile.TileContext(nc) as tc:
        with (
            tc.tile_pool(name="wgt", bufs=1) as wgt,
            tc.tile_pool(name="scrp", bufs=2) as scrp,
            tc.tile_pool(name="ps", bufs=8, space="PSUM") as ps,
        ):
            cst_sb = wgt.tile([1, HIDDEN + 128], bf16, tag="cst")
            gt_sb = wgt.tile([128, NB], f32, tag="gt")
            wu = wgt.tile([128, 512], bf16, tag="wu")
            w1h = [wgt.tile([128, 2048], bf16, tag=f"w1h{h}", name=f"w1h{h}")
                   for h in range(HH)]
            xsb = wgt.tile([128, 4096], bf16, tag="xsb")
            ubt = wgt.tile([128, 8192], bf16, tag="ubt")
            hr = [wgt.tile([128, 1024], bf16, tag=f"hr{b}", name=f"hr{b}")
                  for b in range(NB)]
            outp = wgt.tile([128, NB], f32, tag="outp")
            xb = [xsb[:, b * 512:(b + 1) * 512] for b in range(NB)]
            ub = [ubt[:, b * 1024:(b + 1) * 1024] for b in range(NB)]

            ones_ap = cst_sb[:, HIDDEN:HIDDEN + 128]      # [1, 128]

            # DVE: warmup source (no DMA deps)
            nc.vector.memset(wu[:], 0.5)

            # sync queue (SP): cst first (bias MMs need it), then W1 in
            # consumption order, then U blocks 0-3, then g
            nc.sync.dma_start(cst_sb[:], cstD.ap())
            nc.sync.dma_start(w1h[0][:, 0:1024], w1aD.ap())
            nc.sync.dma_start(w1h[0][:, 1024:2048], w1bD.ap())
            nc.sync.dma_start(w1h[1][:], w1cD.ap())
            nc.sync.dma_start(ubt[:, 0:2048], uqD[0].ap())
            nc.sync.dma_start(ubt[:, 2048:4096], uqD[1].ap())
            nc.sync.dma_start(gt_sb[:], gtD.ap())

            # scalar queue (Activation): x in consumption order, then U 4-7;
            # only 5 triggers so the engine is free before the relu acts
            nc.scalar.dma_start(xsb[:, 0:1536], xqD[0].ap())
            nc.scalar.dma_start(xsb[:, 1536:3072], xqD[1].ap())
            nc.scalar.dma_start(xsb[:, 3072:4096], xqD[2].ap())
            nc.scalar.dma_start(ubt[:, 4096:6144], uqD[2].ap())
            nc.scalar.dma_start(ubt[:, 6144:8192], uqD[3].ap())

            # PE warmup: no DMA deps at all -> runs right after the
            # framework barrier, ramping the PE p-state
            wu_ps = ps.tile([128, 512], f32, tag="ps", bufs=8, name="wu_ps")
            for _ in range(5):
                nc.tensor.matmul(wu_ps[:], wu[:, 0:128], wu[:],
                                 start=True, stop=True)

            def bias_mm(psb, h):
                # psum[r, c] = b1[h*512 + c] for every row r (K=1 matmul)
                nc.tensor.matmul(psb[:], ones_ap,
                                 cst_sb[:, h * 512:(h + 1) * 512],
                                 start=True, stop=False)

            def data_mms(psb, b, h):
                for k in range(KX):
                    nc.tensor.matmul(
                        psb[:], xb[b][:, k * 128:(k + 1) * 128],
                        w1h[h][:, k * 512:(k + 1) * 512],
                        start=False, stop=(k == KX - 1))

            # sweep 1 (hidden half 0): all bias MMs first -- they only
            # need cst, so they execute during the big-DMA lead-in window
            ps1 = [ps.tile([128, 512], f32, tag="ps", bufs=8, name=f"ps0_{b}")
                   for b in range(NB)]
            for b in range(NB):
                bias_mm(ps1[b], 0)
            for b in range(NB):
                data_mms(ps1[b], b, 0)
                nc.scalar.activation(hr[b][:, 0:512], ps1[b][:], AF.Relu)

            # sweep 2 (hidden half 1) + fused reduce per block:
            # delta[128,1] = sum_h hr*U + g  (g seeds the reduction)
            for b in range(NB):
                psb = ps.tile([128, 512], f32, tag="ps", bufs=8, name=f"ps1_{b}")
                bias_mm(psb, 1)
                data_mms(psb, b, 1)
                nc.scalar.activation(hr[b][:, 512:1024], psb[:], AF.Relu)
                scr = scrp.tile([128, 1024], bf16, tag="scr", name=f"scr{b}")
                nc.vector.tensor_tensor(scr[:], hr[b][:], ub[b][:], OP.mult)
                red = scrp.tile([128, 1], f32, tag="red", name=f"red{b}")
                nc.vector.tensor_reduce(
                    red[:], scr[:], mybir.AxisListType.X, OP.add)
                nc.vector.tensor_tensor(
                    outp[:, b:b + 1], red[:], gt_sb[:, b:b + 1], OP.add)

            nc.sync.dma_start(outD.ap(), outp[:])

    nc.compile()
    return nc


def _get_nc():
    if "nc" not in _NC_CACHE:
        import concourse.bacc as bacc
        import concourse.mybir as mybir
        from concourse import tile
        _NC_CACHE["nc"] = _build(bacc.Bacc, mybir, tile)
    return _NC_CACHE["nc"]


def kernel(x_samples, y_idx, W1, b1, W2, b2):
    import ml_dtypes
    from concourse.bass_utils import run_bass_kernel_spmd

    bf16 = ml_dtypes.bfloat16
    x = np.ascontiguousarray(np.asarray(x_samples, dtype=np.float32))
    y = np.asarray(y_idx).astype(np.int64).reshape(-1)
    W1 = np.ascontiguousarray(np.asarray(W1, dtype=np.float32))
    b1 = np.asarray(b1, dtype=np.float32).reshape(-1)
    W2 = np.ascontiguousarray(np.asarray(W2, dtype=np.float32))
    b2 = np.asarray(b2, dtype=np.float32).reshape(-1)

    # global label histogram; fold the softmax-cancelled negative term
    c = np.bincount(y, minlength=Y_DIM).astype(np.float64)
    v = (W2.astype(np.float64) @ c / N).astype(np.float32)     # [H]
    beta = np.float32((b2.astype(np.float64) @ c) / N)
    g_full = (b2[y] - beta).astype(np.float32)                 # [N]

    # U columns, transposed: URt[i, :] = W2[:, y_i] - v
    W2pT = np.ascontiguousarray(W2.T - v[None, :])             # [Y, H]
    W2pT_bf = W2pT.astype(bf16)

    # W1 device layout (shared across cores)
    w1r = W1.reshape(KX, 128, HH, 512).astype(bf16)            # k,p,h,c
    w1a = np.ascontiguousarray(
        w1r[0:2, :, 0].transpose(1, 0, 2).reshape(128, 1024))
    w1b = np.ascontiguousarray(
        w1r[2:4, :, 0].transpose(1, 0, 2).reshape(128, 1024))
    w1c = np.ascontiguousarray(
        w1r[:, :, 1].transpose(1, 0, 2).reshape(128, 2048))
    cst = np.concatenate(
        [b1, np.ones(128, np.float32)]).astype(bf16).reshape(1, -1)

    x_bf = x.astype(bf16)
    in_maps = []
    for m in range(N_CORES):
        sl = slice(m * N_LOC, (m + 1) * N_LOC)
        y_loc = y[sl]
        ur = W2pT_bf[y_loc]                                    # [1024, H]
        im = {"w1a": w1a, "w1b": w1b, "w1c": w1c, "cst": cst,
              "gt": np.ascontiguousarray(
                  g_full[sl].reshape(NB, 128).T)}
        # xs[p, b*512 + k*128 + r] = x[row0 + b*128 + r, k*128 + p]
        xs = np.ascontiguousarray(
            x_bf[sl].reshape(NB, 128, KX, 128)
            .transpose(3, 0, 2, 1).reshape(128, 4096))
        im["xq0"] = np.ascontiguousarray(xs[:, 0:1536])
        im["xq1"] = np.ascontiguousarray(xs[:, 1536:3072])
        im["xq2"] = np.ascontiguousarray(xs[:, 3072:4096])
        # uq[p, b*1024 + h] = U[h, row0 + b*128 + p]
        us = np.ascontiguousarray(
            ur.reshape(NB, 128, HIDDEN).transpose(1, 0, 2)
            .reshape(128, 8192))
        for i in range(4):
            im[f"uq{i}"] = np.ascontiguousarray(
                us[:, i * 2048:(i + 1) * 2048])
        in_maps.append(im)

    nc = _get_nc()
    res = run_bass_kernel_spmd(nc, in_maps, core_ids=list(range(N_CORES)))
    # out[p, blk] holds row blk*128+p of the core's 1024 rows
    return np.concatenate(
        [res.results[m]["out"].T.reshape(-1) for m in range(N_CORES)]
    ).astype(np.float32)
